# revision 8
# baseline (speedup 1.0000x reference)
"""Trainium2 Bass kernel for nn_MentionPrunerSpanBert (topk_masking).

Sharding: 8 cores = 4 docs x 2 T-halves. Each core scores its 3840 spans with
the MLP on the tensor engine in float32r (full PE rate); the 15KB score halves
are reshard-exchanged on the host between two launches; every core then runs
the selection kernel for its doc: exact fp32 re-scoring of a narrow score band
around the top-K threshold (so top-k indices match full-fp32 bit-for-bit),
kth-largest thresholding on GPSIMD, stream-compaction to sorted top-k indices,
indirect-DMA gathers for f_vecs/f_scores/f_begin/f_end, and mask building.
"""
import numpy as np

import concourse.bacc as bacc
import concourse.bass as bass
import concourse.mybir as mybir
from concourse.tile import TileContext

f32 = mybir.dt.float32
f32r = mybir.dt.float32r
i32 = mybir.dt.int32
u32 = mybir.dt.uint32
Alu = mybir.AluOpType
Act = mybir.ActivationFunctionType

B, T, L, D, H = 4, 512, 15, 2048, 1024
K = 205                 # ceil(T * 0.4)
NSP = T * L             # 7680 spans per doc
NSPH = NSP // 2         # 3840 spans per half (per core)
NS = 256                # span chunk for the MLP
KT1, KT2, MT = D // 128, H // 128, H // 128
EPS_BAND = 8e-3         # half-width of the fp32-recompute band (fp32r absmax err ~1e-3)
SENT = 8001.0           # compaction pad value; 8000 after -1, > 7679 -> OOB-skipped
N_CORES = 8

_CACHE = {}


# ---------------------------------------------------------------- kernel 1: MLP
def _emit_mlp(nc, xt_dram, mask_dram, wts, s_out, nspans, ns, variant, pools):
    """Score `nspans` spans: s = mlp(x) + b3 - (1-mask)*1e4  -> s_out [1, nspans]."""
    w1_t, w2_t, w3_t, b1_t, b2_t, b3_t = wts
    xp, hp, psp, smp = pools
    for c in range(nspans // ns):
        c0 = c * ns
        xt_c = xp.tile([128, KT1, ns], f32 if variant == "fp32" else f32r, tag="xt")
        nc.gpsimd.dma_start(out=xt_c[:], in_=xt_dram[:, c0:c0 + ns].rearrange("(k p) n -> p k n", p=128))
        h1 = hp.tile([128, MT, ns], f32 if variant == "fp32" else f32r, tag="h1")
        for m in range(MT):
            acc = psp.tile([128, ns], f32, tag="l1ps")
            for k in range(KT1):
                nc.tensor.matmul(acc[:], w1_t[:, k, m * 128:(m + 1) * 128], xt_c[:, k, :],
                                 start=(k == 0), stop=(k == KT1 - 1))
            nc.scalar.activation(h1[:, m, :], acc[:], Act.Relu, bias=b1_t[:, m:m + 1])
        h2 = hp.tile([128, MT, ns], f32 if variant == "fp32" else f32r, tag="h2")
        for g in range(MT):
            acc = psp.tile([128, ns], f32, tag="l2ps")
            for k in range(KT2):
                nc.tensor.matmul(acc[:], w2_t[:, k, g * 128:(g + 1) * 128], h1[:, k, :],
                                 start=(k == 0), stop=(k == KT2 - 1))
            nc.scalar.activation(h2[:, g, :], acc[:], Act.Relu, bias=b2_t[:, g:g + 1])
        acc3 = psp.tile([1, ns], f32, tag="l3ps")
        for k in range(KT2):
            nc.tensor.matmul(acc3[:], w3_t[:, k:k + 1], h2[:, k, :],
                             start=(k == 0), stop=(k == KT2 - 1))
        m_c = smp.tile([1, ns], f32, tag="mc")
        nc.sync.dma_start(out=m_c[:], in_=mask_dram[c0:c0 + ns].rearrange("(o n) -> o n", o=1))
        pen = smp.tile([1, ns], f32, tag="pen")
        nc.vector.tensor_scalar(pen[:], m_c[:], 1.0, 10000.0, op0=Alu.subtract, op1=Alu.mult)
        nc.vector.tensor_tensor(out=pen[:], in0=pen[:], in1=acc3[:], op=Alu.add)
        nc.vector.tensor_scalar(s_out[:, c0:c0 + ns], pen[:], b3_t[:], None, op0=Alu.add)


def _load_weights(nc, pool, w1, w2, w3, b1, b2, b3, variant):
    mmdt = f32 if variant == "fp32" else f32r
    w1_t = pool.tile([128, KT1, H], mmdt, tag="w1")
    w2_t = pool.tile([128, KT2, H], mmdt, tag="w2")
    w3_t = pool.tile([128, KT2], mmdt, tag="w3")
    # chunked weight loads so the first matmuls can start early
    for k in range(KT1):
        nc.gpsimd.dma_start(out=w1_t[:, k, :], in_=w1[k * 128:(k + 1) * 128, :])
    for k in range(KT2):
        nc.gpsimd.dma_start(out=w2_t[:, k, :], in_=w2[k * 128:(k + 1) * 128, :])
    nc.gpsimd.dma_start(out=w3_t[:], in_=w3.rearrange("(k p) o -> p (k o)", p=128))
    b1_t = pool.tile([128, MT], f32, tag="b1")
    b2_t = pool.tile([128, MT], f32, tag="b2")
    b3_t = pool.tile([1, 1], f32, tag="b3")
    nc.sync.dma_start(out=b1_t[:], in_=b1.rearrange("(m p) -> p m", p=128))
    nc.sync.dma_start(out=b2_t[:], in_=b2.rearrange("(m p) -> p m", p=128))
    nc.sync.dma_start(out=b3_t[:], in_=b3.rearrange("(o n) -> o n", o=1))
    return w1_t, w2_t, w3_t, b1_t, b2_t, b3_t


def _build_nc1():
    nc = bacc.Bacc("TRN2", target_bir_lowering=False, debug=False,
                   enable_asserts=False, num_devices=N_CORES)
    xt = nc.dram_tensor("xt", [D, NSPH], f32, kind="ExternalInput").ap()
    msk = nc.dram_tensor("msk", [NSPH], f32, kind="ExternalInput").ap()
    w1 = nc.dram_tensor("w1", [D, H], f32, kind="ExternalInput").ap()
    w2 = nc.dram_tensor("w2", [H, H], f32, kind="ExternalInput").ap()
    w3 = nc.dram_tensor("w3", [H, 1], f32, kind="ExternalInput").ap()
    b1 = nc.dram_tensor("b1", [H], f32, kind="ExternalInput").ap()
    b2 = nc.dram_tensor("b2", [H], f32, kind="ExternalInput").ap()
    b3 = nc.dram_tensor("b3", [1], f32, kind="ExternalInput").ap()
    s_out = nc.dram_tensor("s_half", [NSPH], f32, kind="ExternalOutput").ap()
    with TileContext(nc) as tc:
        with tc.tile_pool(name="w", bufs=1) as wp, \
             tc.tile_pool(name="x", bufs=2) as xp, \
             tc.tile_pool(name="h", bufs=2) as hp, \
             tc.tile_pool(name="sm", bufs=2) as smp, \
             tc.tile_pool(name="ps", bufs=2, space="PSUM") as psp:
            wts = _load_weights(nc, wp, w1, w2, w3, b1, b2, b3, "fp32r")
            s_sb = wp.tile([1, NSPH], f32, tag="s")
            _emit_mlp(nc, xt, msk, wts, s_sb, NSPH, NS, "fp32r", (xp, hp, psp, smp))
            nc.sync.dma_start(out=s_out.rearrange("(o n) -> o n", o=1), in_=s_sb[:])
    nc.compile()
    return nc


def _emit_bisect(nc, lp, psb, s_tile, ones_f, ones128, lo, hi, niter, target):
    """Dataflow bisection on [128,F] scores: returns lo [128,1] (replicated)
    with count(s >= lo) >= target after `niter` halvings of [lo, hi)."""
    F = s_tile.shape[-1]
    for _ in range(niter):
        mid = lp.tile([128, 1], f32, tag="mid")
        nc.vector.tensor_tensor(out=mid[:], in0=lo[:], in1=hi[:], op=Alu.add)
        nc.vector.tensor_scalar(mid[:], mid[:], 0.5, None, op0=Alu.mult)
        cmp = lp.tile([128, F], f32, tag="cmp")
        pc = lp.tile([128, 1], f32, tag="pc")
        nc.vector.scalar_tensor_tensor(out=cmp[:], in0=s_tile[:], scalar=mid[:], in1=ones_f[:],
                                       op0=Alu.is_ge, op1=Alu.mult, accum_out=pc[:])
        tot_ps = psb.tile([128, 1], f32, tag="tot")
        nc.tensor.matmul(tot_ps[:], ones128[:], pc[:], start=True, stop=True)
        pred = lp.tile([128, 1], f32, tag="pred")
        nc.vector.tensor_scalar(pred[:], tot_ps[:], target - 0.5, None, op0=Alu.is_ge)
        d1 = lp.tile([128, 1], f32, tag="d1")
        nc.vector.tensor_tensor(out=d1[:], in0=mid[:], in1=lo[:], op=Alu.subtract)
        nc.vector.tensor_tensor(out=d1[:], in0=d1[:], in1=pred[:], op=Alu.mult)
        d2 = lp.tile([128, 1], f32, tag="d2")
        nc.vector.tensor_tensor(out=d2[:], in0=hi[:], in1=mid[:], op=Alu.subtract)
        nc.vector.tensor_tensor(out=d2[:], in0=d2[:], in1=pred[:], op=Alu.mult)
        lo_n = lp.tile([128, 1], f32, tag="lo")
        hi_n = lp.tile([128, 1], f32, tag="hi")
        nc.vector.tensor_tensor(out=lo_n[:], in0=lo[:], in1=d1[:], op=Alu.add)
        nc.vector.tensor_tensor(out=hi_n[:], in0=mid[:], in1=d2[:], op=Alu.add)
        lo, hi = lo_n, hi_n
    return lo


# ------------------------------------------------------------ kernel 2: select
def _build_nc2():
    nc = bacc.Bacc("TRN2", target_bir_lowering=False, debug=False,
                   enable_asserts=False, num_devices=N_CORES)
    s_in = nc.dram_tensor("s_in", [NSP], f32, kind="ExternalInput").ap()
    x_in = nc.dram_tensor("x_in", [NSP, D], f32, kind="ExternalInput").ap()
    w1 = nc.dram_tensor("w1", [D, H], f32, kind="ExternalInput").ap()
    w2 = nc.dram_tensor("w2", [H, H], f32, kind="ExternalInput").ap()
    w3 = nc.dram_tensor("w3", [H, 1], f32, kind="ExternalInput").ap()
    b1 = nc.dram_tensor("b1", [H], f32, kind="ExternalInput").ap()
    b2 = nc.dram_tensor("b2", [H], f32, kind="ExternalInput").ap()
    b3 = nc.dram_tensor("b3", [1], f32, kind="ExternalInput").ap()
    seq = nc.dram_tensor("seq", [1, 1], i32, kind="ExternalInput").ap()
    bt = nc.dram_tensor("bt", [NSP, 2], i32, kind="ExternalInput").ap()

    scores_out = nc.dram_tensor("scores_out", [NSP], f32, kind="ExternalOutput").ap()
    tidx_out = nc.dram_tensor("tidx_out", [256], i32, kind="ExternalOutput").ap()
    fvec_out = nc.dram_tensor("fvec_out", [256, D], f32, kind="ExternalOutput").ap()
    fs_out = nc.dram_tensor("fs_out", [256, 1], f32, kind="ExternalOutput").ap()
    fbt_out = nc.dram_tensor("fbt_out", [256, 2], i32, kind="ExternalOutput").ap()
    sl_out = nc.dram_tensor("sl_out", [1, 1], i32, kind="ExternalOutput").ap()
    sq_out = nc.dram_tensor("sq_out", [K, K], f32, kind="ExternalOutput").ap()
    tri_out = nc.dram_tensor("tri_out", [K, K], f32, kind="ExternalOutput").ap()

    QUANT = 1.0 - (float(K) - 0.5) / (NSP - 1)

    with TileContext(nc) as tc:
        with tc.tile_pool(name="w", bufs=1) as wp, \
             tc.tile_pool(name="sb", bufs=1) as sb, \
             tc.tile_pool(name="g", bufs=2) as gp, \
             tc.tile_pool(name="lp", bufs=2) as lp, \
             tc.tile_pool(name="dram", bufs=1, space="DRAM") as dp, \
             tc.tile_pool(name="ps", bufs=1, space="PSUM") as psp, \
             tc.tile_pool(name="psb", bufs=2, space="PSUM") as psb, \
             tc.tile_pool(name="ps2", bufs=2, space="PSUM") as psp2:
            wts = _load_weights(nc, wp, w1, w2, w3, b1, b2, b3, "fp32")
            w1_t, w2_t, w3_t, b1_t, b2_t, b3_t = wts

            ones128 = sb.tile([128, 128], f32)
            nc.vector.memset(ones128[:], 1.0)
            ones60 = sb.tile([128, NSP // 128], f32)
            nc.vector.memset(ones60[:], 1.0)

            # -- coarse threshold of the raw (fp32r) scores via bisection
            s1 = sb.tile([128, NSP // 128], f32)
            nc.sync.dma_start(out=s1[:], in_=s_in.rearrange("(p f) -> p f", p=128))
            gmax = sb.tile([1, 1], f32)
            nc.gpsimd.tensor_reduce(out=gmax[:], in_=s1[:], axis=mybir.AxisListType.XYZWC, op=Alu.max)
            hi0_ps = psb.tile([128, 1], f32, tag="tot")
            nc.tensor.matmul(hi0_ps[:], ones128[0:1, :], gmax[:], start=True, stop=True)
            hi0 = sb.tile([128, 1], f32)
            nc.vector.tensor_scalar(hi0[:], hi0_ps[:], 1.0, None, op0=Alu.add)
            lo0 = sb.tile([128, 1], f32)
            nc.vector.tensor_scalar(lo0[:], hi0[:], 20002.0, None, op0=Alu.subtract)
            thr = _emit_bisect(nc, lp, psb, s1, ones60, ones128, lo0, hi0, 26, float(K))

            # -- band = spans within EPS_BAND of the threshold (layout [16,480])
            s3 = sb.tile([16, NSP // 16], f32)
            nc.sync.dma_start(out=s3[:], in_=s_in.rearrange("(f p) -> p f", p=16))
            blo = sb.tile([128, 1], f32)
            bhi = sb.tile([128, 1], f32)
            nc.vector.tensor_scalar(blo[:], thr[:], EPS_BAND, None, op0=Alu.subtract)
            nc.vector.tensor_scalar(bhi[:], thr[:], EPS_BAND, None, op0=Alu.add)
            blo, bhi = blo[0:16, :], bhi[0:16, :]
            bm = sb.tile([16, NSP // 16], f32)
            bm2 = sb.tile([16, NSP // 16], f32)
            nc.vector.tensor_scalar(bm[:], s3[:], blo[:], None, op0=Alu.is_ge)
            nc.vector.tensor_scalar(bm2[:], s3[:], bhi[:], None, op0=Alu.is_le)
            nc.vector.tensor_tensor(out=bm[:], in0=bm[:], in1=bm2[:], op=Alu.mult)
            iota1 = sb.tile([16, NSP // 16], f32)
            nc.gpsimd.iota(iota1[:], pattern=[[16, NSP // 16]], base=1, channel_multiplier=1,
                           allow_small_or_imprecise_dtypes=True)
            bidx = sb.tile([16, NSP // 16 + 8], f32)
            nc.vector.tensor_tensor(out=bidx[:, :NSP // 16], in0=iota1[:], in1=bm[:], op=Alu.mult)
            nc.vector.tensor_scalar(bidx[:, :NSP // 16], bidx[:, :NSP // 16], 1.0, None, op0=Alu.subtract)
            nc.vector.memset(bidx[:, NSP // 16:], SENT)
            bcg = sb.tile([16, 8], f32)
            bnf = sb.tile([1, 1], u32)
            nc.gpsimd.sparse_gather(bcg[:], bidx[:], num_found=bnf[:])
            bti = sb.tile([16, 8], i32)
            nc.vector.tensor_copy(out=bti[:], in_=bcg[:])
            bscr = dp.tile([128], i32)
            nc.sync.dma_start(out=bscr[:].rearrange("(f p) -> p f", p=16), in_=bti[:])
            bIdx = sb.tile([128, 1], i32)
            nc.sync.dma_start(out=bIdx[:], in_=bscr[:].rearrange("(p o) -> p o", o=1))

            # -- gather band rows of X, transpose, fp32 re-score
            xb = gp.tile([128, D], f32, tag="xb")
            nc.gpsimd.indirect_dma_start(
                out=xb[:], out_offset=None, in_=x_in[:],
                in_offset=bass.IndirectOffsetOnAxis(ap=bIdx[:, 0:1], axis=0),
                bounds_check=NSP - 1, oob_is_err=False)
            ident = sb.tile([128, 128], f32)
            ones = sb.tile([128, 128], f32)
            nc.vector.memset(ones[:], 1.0)
            nc.gpsimd.affine_select(ident[:], ones[:], pattern=[[-1, 128]],
                                    compare_op=Alu.is_equal, fill=0.0, base=0, channel_multiplier=1)
            xtb = gp.tile([128, KT1, 128], f32, tag="xtb")
            for k in range(KT1):
                tp_ps = psp.tile([128, 128], f32, tag="tp")
                nc.tensor.transpose(tp_ps[:], xb[:, k * 128:(k + 1) * 128], ident[:])
                nc.vector.tensor_copy(out=xtb[:, k, :], in_=tp_ps[:])
            h1b = gp.tile([128, MT, 128], f32, tag="h1b")
            for m in range(MT):
                acc = psp2.tile([128, 128], f32, tag="bl1")
                for k in range(KT1):
                    nc.tensor.matmul(acc[:], w1_t[:, k, m * 128:(m + 1) * 128], xtb[:, k, :],
                                     start=(k == 0), stop=(k == KT1 - 1))
                nc.scalar.activation(h1b[:, m, :], acc[:], Act.Relu, bias=b1_t[:, m:m + 1])
            h2b = gp.tile([128, MT, 128], f32, tag="h2b")
            for g in range(MT):
                acc = psp.tile([128, 128], f32, tag="bl2")
                for k in range(KT2):
                    nc.tensor.matmul(acc[:], w2_t[:, k, g * 128:(g + 1) * 128], h1b[:, k, :],
                                     start=(k == 0), stop=(k == KT2 - 1))
                nc.scalar.activation(h2b[:, g, :], acc[:], Act.Relu, bias=b2_t[:, g:g + 1])
            acc3 = psp.tile([1, 128], f32, tag="bl3")
            for k in range(KT2):
                nc.tensor.matmul(acc3[:], w3_t[:, k:k + 1], h2b[:, k, :],
                                 start=(k == 0), stop=(k == KT2 - 1))
            sb_band = sb.tile([1, 128], f32)
            nc.vector.tensor_scalar(sb_band[:], acc3[:], b3_t[:], None, op0=Alu.add)
            # scatter needs values per partition: transpose [1,128] -> [128,1]
            sbt_ps = psp.tile([128, 1], f32, tag="bl3")
            nc.tensor.transpose(sbt_ps[:], sb_band[:], ident[0:1, 0:1])
            sbv = sb.tile([128, 1], f32)
            nc.vector.tensor_copy(out=sbv[:], in_=sbt_ps[:])

            # -- patch scores in DRAM
            s_patch = dp.tile([NSP], f32)
            nc.sync.dma_start(out=s_patch[:], in_=s_in[:])
            nc.gpsimd.indirect_dma_start(
                out=s_patch[:].rearrange("(n o) -> n o", o=1),
                out_offset=bass.IndirectOffsetOnAxis(ap=bIdx[:, 0:1], axis=0),
                in_=sbv[:], in_offset=None,
                bounds_check=NSP - 1, oob_is_err=False)
            nc.sync.dma_start(out=scores_out[:], in_=s_patch[:])

            # -- final threshold + selection on patched scores
            s1p = sb.tile([128, NSP // 128], f32)
            nc.sync.dma_start(out=s1p[:], in_=s_patch[:].rearrange("(p f) -> p f", p=128))
            lo2 = sb.tile([128, 1], f32)
            hi2 = sb.tile([128, 1], f32)
            nc.vector.tensor_scalar(lo2[:], thr[:], 0.02, None, op0=Alu.subtract)
            nc.vector.tensor_scalar(hi2[:], thr[:], 0.02, None, op0=Alu.add)
            thrf = _emit_bisect(nc, lp, psb, s1p, ones60, ones128, lo2, hi2, 22, float(K))
            s3p = sb.tile([16, NSP // 16], f32)
            nc.sync.dma_start(out=s3p[:], in_=s_patch[:].rearrange("(f p) -> p f", p=16))
            m3 = sb.tile([16, NSP // 16], f32)
            nc.vector.tensor_scalar(m3[:], s3p[:], thrf[0:16, :], None, op0=Alu.is_ge)
            midx = sb.tile([16, NSP // 16 + 16], f32)
            nc.vector.tensor_tensor(out=midx[:, :NSP // 16], in0=iota1[:], in1=m3[:], op=Alu.mult)
            nc.vector.tensor_scalar(midx[:, :NSP // 16], midx[:, :NSP // 16], 1.0, None, op0=Alu.subtract)
            nc.vector.memset(midx[:, NSP // 16:], SENT)
            cg = sb.tile([16, 16], f32)
            nf = sb.tile([1, 1], u32)
            nc.gpsimd.sparse_gather(cg[:], midx[:], num_found=nf[:])
            ti = sb.tile([16, 16], i32)
            nc.vector.tensor_copy(out=ti[:], in_=cg[:])
            nc.sync.dma_start(out=tidx_out.rearrange("(f p) -> p f", p=16), in_=ti[:])
            tscr = dp.tile([256], i32)
            nc.sync.dma_start(out=tscr[:].rearrange("(f p) -> p f", p=16), in_=ti[:])
            idxA = sb.tile([128, 1], i32)
            idxB = sb.tile([128, 1], i32)
            nc.sync.dma_start(out=idxA[:], in_=tscr[0:128].rearrange("(p o) -> p o", o=1))
            nc.sync.dma_start(out=idxB[:], in_=tscr[128:256].rearrange("(p o) -> p o", o=1))

            # -- gathers
            for half, idx in ((0, idxA), (1, idxB)):
                fv = gp.tile([128, D], f32, tag="fv")
                nc.gpsimd.indirect_dma_start(
                    out=fv[:], out_offset=None, in_=x_in[:],
                    in_offset=bass.IndirectOffsetOnAxis(ap=idx[:, 0:1], axis=0),
                    bounds_check=NSP - 1, oob_is_err=False)
                nc.sync.dma_start(out=fvec_out[half * 128:(half + 1) * 128, :], in_=fv[:])
                fs = gp.tile([128, 1], f32, tag="fs")
                nc.gpsimd.indirect_dma_start(
                    out=fs[:], out_offset=None, in_=s_patch[:].rearrange("(n o) -> n o", o=1),
                    in_offset=bass.IndirectOffsetOnAxis(ap=idx[:, 0:1], axis=0),
                    bounds_check=NSP - 1, oob_is_err=False)
                nc.sync.dma_start(out=fs_out[half * 128:(half + 1) * 128, :], in_=fs[:])
                fb = gp.tile([128, 2], i32, tag="fb")
                nc.gpsimd.indirect_dma_start(
                    out=fb[:], out_offset=None, in_=bt[:],
                    in_offset=bass.IndirectOffsetOnAxis(ap=idx[:, 0:1], axis=0),
                    bounds_check=NSP - 1, oob_is_err=False)
                nc.sync.dma_start(out=fbt_out[half * 128:(half + 1) * 128, :], in_=fb[:])

            # -- span length + masks
            sl_i = sb.tile([1, 1], i32)
            nc.sync.dma_start(out=sl_i[:], in_=seq[:])
            sl_f = sb.tile([1, 1], f32)
            nc.vector.tensor_copy(out=sl_f[:], in_=sl_i[:])
            nc.vector.tensor_scalar(sl_f[:], sl_f[:], 0.4, 0.49, op0=Alu.mult, op1=Alu.add)
            slq_i = sb.tile([1, 1], i32)
            nc.vector.tensor_copy(out=slq_i[:], in_=sl_f[:])  # round-to-nearest == ceil(0.4n)
            nc.sync.dma_start(out=sl_out[:], in_=slq_i[:])
            slq_f = sb.tile([1, 1], f32)
            nc.vector.tensor_copy(out=slq_f[:], in_=slq_i[:])
            iota_k = sb.tile([1, K], f32)
            nc.gpsimd.iota(iota_k[:], pattern=[[1, K]], base=0, channel_multiplier=0,
                           allow_small_or_imprecise_dtypes=True)
            valid = sb.tile([1, K], f32)
            nc.vector.tensor_scalar(valid[:], iota_k[:], slq_f[:], None, op0=Alu.is_lt)
            sqA_ps = psp.tile([128, K], f32, tag="tp")
            sqB_ps = psp.tile([77, K], f32, tag="bl2")
            nc.tensor.matmul(sqA_ps[:], valid[:, 0:128], valid[:], start=True, stop=True)
            nc.tensor.matmul(sqB_ps[:], valid[:, 128:K], valid[:], start=True, stop=True)
            sqA = sb.tile([128, K], f32)
            sqB = sb.tile([77, K], f32)
            nc.vector.tensor_copy(out=sqA[:], in_=sqA_ps[:])
            nc.vector.tensor_copy(out=sqB[:], in_=sqB_ps[:])
            nc.sync.dma_start(out=sq_out[0:128, :], in_=sqA[:])
            nc.sync.dma_start(out=sq_out[128:K, :], in_=sqB[:])
            triA = sb.tile([128, K], f32)
            triB = sb.tile([77, K], f32)
            nc.gpsimd.affine_select(triA[:], sqA[:], pattern=[[-1, K]], compare_op=Alu.is_ge,
                                    fill=0.0, base=0, channel_multiplier=1)
            nc.gpsimd.affine_select(triB[:], sqB[:], pattern=[[-1, K]], compare_op=Alu.is_ge,
                                    fill=0.0, base=128, channel_multiplier=1)
            nc.sync.dma_start(out=tri_out[0:128, :], in_=triA[:])
            nc.sync.dma_start(out=tri_out[128:K, :], in_=triB[:])
    nc.compile()
    return nc


# ------------------------------------------------------------------ PJRT runner
class _Runner:
    """Build a jitted sharded executor for a finalized Bass module once."""

    def __init__(self, nc, n_cores):
        import jax
        from jax.sharding import Mesh, PartitionSpec
        from jax.experimental.shard_map import shard_map
        from concourse.bass2jax import (_bass_exec_p, install_neuronx_cc_hook,
                                        partition_id_tensor)
        install_neuronx_cc_hook()
        self.jax = jax
        self.n_cores = n_cores
        partition_name = nc.partition_id_tensor.name if nc.partition_id_tensor else None
        in_names, out_names, out_avals, zero_outs = [], [], [], []
        for alloc in nc.m.functions[0].allocations:
            if not isinstance(alloc, mybir.MemoryLocationSet):
                continue
            name = alloc.memorylocations[0].name
            if alloc.kind == "ExternalInput":
                if name != partition_name:
                    in_names.append(name)
            elif alloc.kind == "ExternalOutput":
                out_names.append(name)
                out_avals.append(jax.core.ShapedArray(tuple(alloc.tensor_shape),
                                                      mybir.dt.np(alloc.dtype)))
                zero_outs.append(np.zeros(tuple(alloc.tensor_shape), mybir.dt.np(alloc.dtype)))
        self.in_names, self.out_names = in_names, out_names
        self.out_avals, self.zero_outs = out_avals, zero_outs
        all_in_names = list(in_names) + list(out_names)
        if partition_name is not None:
            all_in_names.append(partition_name)
        n_params, n_outs = len(in_names), len(out_avals)

        def _body(*args):
            operands = list(args)
            if partition_name is not None:
                operands.append(partition_id_tensor())
            return tuple(_bass_exec_p.bind(
                *operands,
                out_avals=tuple(out_avals), in_names=tuple(all_in_names),
                out_names=tuple(out_names), lowering_input_output_aliases=(),
                sim_require_finite=True, sim_require_nnan=True, nc=nc))

        devices = jax.devices()[:n_cores]
        self.mesh = Mesh(np.asarray(devices), ("core",))
        in_specs = (PartitionSpec("core"),) * (n_params + n_outs)
        out_specs = (PartitionSpec("core"),) * n_outs
        self.fn = jax.jit(shard_map(_body, mesh=self.mesh, in_specs=in_specs,
                                    out_specs=out_specs, check_rep=False),
                          keep_unused=True)

    def __call__(self, in_maps):
        n = self.n_cores
        args = [np.concatenate([np.asarray(in_maps[c][name]) for c in range(n)], axis=0)
                for name in self.in_names]
        args += [np.zeros((n * z.shape[0], *z.shape[1:]), z.dtype) for z in self.zero_outs]
        outs = self.fn(*args)
        self.jax.block_until_ready(outs)
        res = []
        for c in range(n):
            res.append({name: np.asarray(outs[i]).reshape(n, *self.out_avals[i].shape)[c]
                        for i, name in enumerate(self.out_names)})
        return res


def _get_runners():
    if "runners" not in _CACHE:
        nc1, nc2 = _build_nc1(), _build_nc2()
        _CACHE["nc1"], _CACHE["nc2"] = nc1, nc2
        _CACHE["runners"] = (_Runner(nc1, N_CORES), _Runner(nc2, N_CORES))
    return _CACHE["runners"]


# ----------------------------------------------------------------------- kernel
def kernel(span_vecs, span_mask, span_begin, span_end, sequence_lengths,
           W1, b1, W2, b2, W3, b3):
    span_vecs = np.asarray(span_vecs, np.float32)
    span_mask = np.asarray(span_mask, np.float32)
    span_begin = np.asarray(span_begin, np.int32)
    span_end = np.asarray(span_end, np.int32)
    sequence_lengths = np.asarray(sequence_lengths, np.int32)
    W1 = np.asarray(W1, np.float32)
    W2 = np.asarray(W2, np.float32)
    W3 = np.asarray(W3, np.float32)
    b1 = np.asarray(b1, np.float32)
    b2 = np.asarray(b2, np.float32)
    b3 = np.asarray(b3, np.float32)

    run1, run2 = _get_runners()
    wmap = {"w1": W1, "w2": W2, "w3": W3, "b1": b1, "b2": b2, "b3": b3}

    # ---- launch 1: MLP scores, data-parallel over (doc, half)
    in1 = []
    for c in range(N_CORES):
        d, h = c // 2, c % 2
        xh = span_vecs[d].reshape(NSP, D)[h * NSPH:(h + 1) * NSPH]
        in1.append({"xt": np.ascontiguousarray(xh.T),
                    "msk": span_mask[d].reshape(NSP)[h * NSPH:(h + 1) * NSPH],
                    **wmap})
    res1 = run1(in1)

    # ---- host reshard: full per-doc score vectors
    s_full = [np.concatenate([res1[2 * d]["s_half"], res1[2 * d + 1]["s_half"]])
              for d in range(B)]

    # ---- launch 2: selection (both cores of a pair run the identical doc)
    in2 = []
    for c in range(N_CORES):
        d = c // 2
        bt = np.stack([span_begin[d].reshape(NSP), span_end[d].reshape(NSP)], axis=1)
        in2.append({"s_in": s_full[d],
                    "x_in": span_vecs[d].reshape(NSP, D),
                    "seq": sequence_lengths[d].reshape(1, 1),
                    "bt": np.ascontiguousarray(bt),
                    **wmap})
    res2 = run2(in2)

    # ---- unshard
    prune_scores = np.stack([res2[2 * d]["scores_out"] for d in range(B)]
                            ).reshape(B, T, L, 1)
    top_idx = np.stack([res2[2 * d]["tidx_out"][:K] for d in range(B)])
    f_vecs = np.stack([res2[2 * d]["fvec_out"][:K] for d in range(B)])
    f_scores = np.stack([res2[2 * d]["fs_out"][:K] for d in range(B)])
    f_begin = np.stack([res2[2 * d]["fbt_out"][:K, 0:1] for d in range(B)])
    f_end = np.stack([res2[2 * d]["fbt_out"][:K, 1:2] for d in range(B)])
    span_lengths = np.stack([res2[2 * d]["sl_out"][0, 0] for d in range(B)])
    square_mask = np.stack([res2[2 * d]["sq_out"] for d in range(B)])
    triangular_mask = np.stack([res2[2 * d]["tri_out"] for d in range(B)])

    return (prune_scores, top_idx.astype(np.int32), f_vecs, f_scores,
            f_begin.astype(np.int32), f_end.astype(np.int32),
            span_lengths.astype(np.int32), square_mask, triangular_mask)


# revision 15
# speedup vs baseline: 1.2974x; 1.2974x over previous
"""Trainium2 Bass kernel for nn_MentionPrunerSpanBert (topk_masking).

Sharding: 8 cores = 4 docs x 2 T-halves. Each core scores its 3840 spans with
the MLP on the tensor engine in float32r (full PE rate); the 15KB score halves
are reshard-exchanged on the host between two launches; every core then runs
the selection kernel for its doc: exact fp32 re-scoring of a narrow score band
around the top-K threshold (so top-k indices match full-fp32 bit-for-bit),
kth-largest thresholding on GPSIMD, stream-compaction to sorted top-k indices,
indirect-DMA gathers for f_vecs/f_scores/f_begin/f_end, and mask building.
"""
import numpy as np

import concourse.bacc as bacc
import concourse.bass as bass
import concourse.mybir as mybir
from concourse.tile import TileContext

f32 = mybir.dt.float32
f32r = mybir.dt.float32r
i32 = mybir.dt.int32
u32 = mybir.dt.uint32
Alu = mybir.AluOpType
Act = mybir.ActivationFunctionType

B, T, L, D, H = 4, 512, 15, 2048, 1024
K = 205                 # ceil(T * 0.4)
NSP = T * L             # 7680 spans per doc
NSPH = NSP // 2         # 3840 spans per half (per core)
NS = 384                # span chunk for the MLP
KT1, KT2, MT = D // 128, H // 128, H // 128
EPS_BAND = 8e-3         # half-width of the fp32-recompute band (fp32r absmax err ~1e-3)
SENT = 8001.0           # compaction pad value; 8000 after -1, > 7679 -> OOB-skipped
N_CORES = 8

_CACHE = {}


# ---------------------------------------------------------------- kernel 1: MLP
def _emit_mlp(nc, xt_dram, mask_dram, wts, s_out, nspans, ns, variant, pools):
    """Score `nspans` spans: s = mlp(x) + b3 - (1-mask)*1e4  -> s_out [1, nspans]."""
    w1_t, w2_t, w3_t, b1_t, b2_t, b3_t = wts
    xp, hp, psp, smp = pools[:4]
    xt_pre = pools[4] if len(pools) > 4 else {}
    for c in range(nspans // ns):
        c0 = c * ns
        if c in xt_pre:
            xt_c = xt_pre[c]
        else:
            xt_c = xp.tile([128, KT1, ns], f32 if variant == "fp32" else f32r, tag="xt")
            nc.sync.dma_start(out=xt_c[:], in_=xt_dram[:, c0:c0 + ns].rearrange("(k p) n -> p k n", p=128))
        h1 = hp.tile([128, MT, ns], f32 if variant == "fp32" else f32r, tag="h1")
        for m in range(MT):
            acc = psp.tile([128, ns], f32, tag="l1ps")
            for k in range(KT1):
                nc.tensor.matmul(acc[:], w1_t[:, k, m * 128:(m + 1) * 128], xt_c[:, k, :],
                                 start=(k == 0), stop=(k == KT1 - 1))
            nc.scalar.activation(h1[:, m, :], acc[:], Act.Relu, bias=b1_t[:, m:m + 1])
        h2 = hp.tile([128, MT, ns], f32 if variant == "fp32" else f32r, tag="h2")
        for g in range(MT):
            acc = psp.tile([128, ns], f32, tag="l2ps")
            for k in range(KT2):
                nc.tensor.matmul(acc[:], w2_t[:, k, g * 128:(g + 1) * 128], h1[:, k, :],
                                 start=(k == 0), stop=(k == KT2 - 1))
            nc.scalar.activation(h2[:, g, :], acc[:], Act.Relu, bias=b2_t[:, g:g + 1])
        acc3 = psp.tile([1, ns], f32, tag="l3ps")
        for k in range(KT2):
            nc.tensor.matmul(acc3[:], w3_t[:, k:k + 1], h2[:, k, :],
                             start=(k == 0), stop=(k == KT2 - 1))
        m_c = smp.tile([1, ns], f32, tag="mc")
        nc.sync.dma_start(out=m_c[:], in_=mask_dram[c0:c0 + ns].rearrange("(o n) -> o n", o=1))
        pen = smp.tile([1, ns], f32, tag="pen")
        nc.vector.tensor_scalar(pen[:], m_c[:], 1.0, 10000.0, op0=Alu.subtract, op1=Alu.mult)
        nc.vector.tensor_tensor(out=pen[:], in0=pen[:], in1=acc3[:], op=Alu.add)
        nc.vector.tensor_scalar(s_out[:, c0:c0 + ns], pen[:], b3_t[:], None, op0=Alu.add)


def _load_weights(nc, pool, w1, w2, w3, b1, b2, b3, variant):
    mmdt = f32 if variant == "fp32" else f32r
    w1_t = pool.tile([128, KT1, H], mmdt, tag="w1")
    w2_t = pool.tile([128, KT2, H], mmdt, tag="w2")
    w3_t = pool.tile([128, KT2], mmdt, tag="w3")
    # chunked weight loads so the first matmuls can start early
    for k in range(KT1):
        nc.sync.dma_start(out=w1_t[:, k, :], in_=w1[k * 128:(k + 1) * 128, :])
    for k in range(KT2):
        nc.sync.dma_start(out=w2_t[:, k, :], in_=w2[k * 128:(k + 1) * 128, :])
    nc.sync.dma_start(out=w3_t[:], in_=w3.rearrange("(k p) o -> p (k o)", p=128))
    b1_t = pool.tile([128, MT], f32, tag="b1")
    b2_t = pool.tile([128, MT], f32, tag="b2")
    b3_t = pool.tile([1, 1], f32, tag="b3")
    nc.sync.dma_start(out=b1_t[:], in_=b1.rearrange("(m p) -> p m", p=128))
    nc.sync.dma_start(out=b2_t[:], in_=b2.rearrange("(m p) -> p m", p=128))
    nc.sync.dma_start(out=b3_t[:], in_=b3.rearrange("(o n) -> o n", o=1))
    return w1_t, w2_t, w3_t, b1_t, b2_t, b3_t


def _build_nc1():
    nc = bacc.Bacc("TRN2", target_bir_lowering=False, debug=False,
                   enable_asserts=False, num_devices=N_CORES)
    xt = nc.dram_tensor("xt", [D, NSPH], f32r, kind="ExternalInput").ap()
    msk = nc.dram_tensor("msk", [NSPH], f32, kind="ExternalInput").ap()
    w1 = nc.dram_tensor("w1", [D, H], f32r, kind="ExternalInput").ap()
    w2 = nc.dram_tensor("w2", [H, H], f32r, kind="ExternalInput").ap()
    w3 = nc.dram_tensor("w3", [H, 1], f32r, kind="ExternalInput").ap()
    b1 = nc.dram_tensor("b1", [H], f32, kind="ExternalInput").ap()
    b2 = nc.dram_tensor("b2", [H], f32, kind="ExternalInput").ap()
    b3 = nc.dram_tensor("b3", [1], f32, kind="ExternalInput").ap()
    s_out = nc.dram_tensor("s_half", [NSPH], f32, kind="ExternalOutput").ap()
    with TileContext(nc) as tc:
        with tc.tile_pool(name="w", bufs=1) as wp, \
             tc.tile_pool(name="x", bufs=2) as xp, \
             tc.tile_pool(name="h", bufs=1) as hp, \
             tc.tile_pool(name="sm", bufs=2) as smp, \
             tc.tile_pool(name="ps", bufs=2, space="PSUM") as psp:
            xt0 = xp.tile([128, KT1, NS], f32r, tag="xt")
            nc.sync.dma_start(out=xt0[:], in_=xt[:, 0:NS].rearrange("(k p) n -> p k n", p=128))
            wts = _load_weights(nc, wp, w1, w2, w3, b1, b2, b3, "fp32r")
            s_sb = wp.tile([1, NSPH], f32, tag="s")
            _emit_mlp(nc, xt, msk, wts, s_sb, NSPH, NS, "fp32r", (xp, hp, psp, smp, {0: xt0}))
            nc.sync.dma_start(out=s_out.rearrange("(o n) -> o n", o=1), in_=s_sb[:])
    nc.compile()
    return nc


def _emit_bisect(nc, lp, psb, s_tile, ones_bf, lo, width, niter, target):
    """Dataflow bisection on [128,F] scores with statically-halving width.
    Invariant: count(s >= lo) >= target > count(s >= lo + w). Returns final
    lo [128,1] (replicated across partitions)."""
    F = s_tile.shape[-1]
    bf16 = mybir.dt.bfloat16
    for it in range(niter):
        w = width / (2.0 ** (it + 1))
        mid = lp.tile([128, 1], f32, tag="mid")
        nc.vector.tensor_scalar(mid[:], lo[:], w, None, op0=Alu.add)
        cmp = lp.tile([128, F], bf16, tag="cmp")
        pc = lp.tile([128, 1], bf16, tag="pc")
        nc.vector.scalar_tensor_tensor(out=cmp[:], in0=s_tile[:], scalar=mid[:], in1=ones_bf[:, 0:F],
                                       op0=Alu.is_ge, op1=Alu.mult, accum_out=pc[:])
        tot_ps = psb.tile([128, 1], f32, tag="tot")
        nc.tensor.matmul(tot_ps[:], ones_bf[:, 0:128], pc[:], start=True, stop=True)
        pred = lp.tile([128, 1], mybir.dt.uint8, tag="pred")
        nc.vector.tensor_scalar(pred[:], tot_ps[:], target - 0.5, None, op0=Alu.is_ge)
        lo_n = lp.tile([128, 1], f32, tag="lo")
        nc.vector.select(lo_n[:], pred[:], mid[:], lo[:])
        lo = lo_n
    return lo


# ------------------------------------------------------------ kernel 2: select
def _build_nc2():
    nc = bacc.Bacc("TRN2", target_bir_lowering=False, debug=False,
                   enable_asserts=False, num_devices=N_CORES)
    s_in = nc.dram_tensor("s_in", [NSP], f32, kind="ExternalInput").ap()
    x_in = nc.dram_tensor("x_in", [NSP, D], f32, kind="ExternalInput").ap()
    w1 = nc.dram_tensor("w1", [D, H], f32, kind="ExternalInput").ap()
    w2 = nc.dram_tensor("w2", [H, H], f32, kind="ExternalInput").ap()
    w3 = nc.dram_tensor("w3", [H, 1], f32, kind="ExternalInput").ap()
    b1 = nc.dram_tensor("b1", [H], f32, kind="ExternalInput").ap()
    b2 = nc.dram_tensor("b2", [H], f32, kind="ExternalInput").ap()
    b3 = nc.dram_tensor("b3", [1], f32, kind="ExternalInput").ap()
    seq = nc.dram_tensor("seq", [1, 1], i32, kind="ExternalInput").ap()
    bt = nc.dram_tensor("bt", [NSP, 2], i32, kind="ExternalInput").ap()

    scores_out = nc.dram_tensor("scores_out", [NSP], f32, kind="ExternalOutput").ap()
    tidx_out = nc.dram_tensor("tidx_out", [256], i32, kind="ExternalOutput").ap()
    fvec_out = nc.dram_tensor("fvec_out", [256, D], f32, kind="ExternalOutput").ap()
    fs_out = nc.dram_tensor("fs_out", [256, 1], f32, kind="ExternalOutput").ap()
    fbt_out = nc.dram_tensor("fbt_out", [256, 2], i32, kind="ExternalOutput").ap()
    sl_out = nc.dram_tensor("sl_out", [1, 1], i32, kind="ExternalOutput").ap()
    sq_out = nc.dram_tensor("sq_out", [K, K], f32, kind="ExternalOutput").ap()
    tri_out = nc.dram_tensor("tri_out", [K, K], f32, kind="ExternalOutput").ap()

    QUANT = 1.0 - (float(K) - 0.5) / (NSP - 1)

    with TileContext(nc) as tc:
        with tc.tile_pool(name="w", bufs=1) as wp, \
             tc.tile_pool(name="sb", bufs=1) as sb, \
             tc.tile_pool(name="g", bufs=2) as gp, \
             tc.tile_pool(name="lp", bufs=2) as lp, \
             tc.tile_pool(name="dram", bufs=1, space="DRAM") as dp, \
             tc.tile_pool(name="ps", bufs=1, space="PSUM") as psp, \
             tc.tile_pool(name="psb", bufs=2, space="PSUM") as psb, \
             tc.tile_pool(name="ps2", bufs=2, space="PSUM") as psp2:
            wts = _load_weights(nc, wp, w1, w2, w3, b1, b2, b3, "fp32")
            w1_t, w2_t, w3_t, b1_t, b2_t, b3_t = wts

            ones_bf = sb.tile([128, 128], mybir.dt.bfloat16)
            nc.vector.memset(ones_bf[:], 1.0)
            ones1 = sb.tile([1, 128], f32)
            nc.vector.memset(ones1[:], 1.0)

            # -- coarse threshold of the raw (fp32r) scores via bisection
            s1 = sb.tile([128, NSP // 128], f32)
            nc.sync.dma_start(out=s1[:], in_=s_in.rearrange("(p f) -> p f", p=128))
            gmax = sb.tile([1, 1], f32)
            nc.gpsimd.tensor_reduce(out=gmax[:], in_=s1[:], axis=mybir.AxisListType.XYZWC, op=Alu.max)
            hi0_ps = psb.tile([128, 1], f32, tag="tot")
            nc.tensor.matmul(hi0_ps[:], ones1[:], gmax[:], start=True, stop=True)
            lo0 = sb.tile([128, 1], f32)
            nc.vector.tensor_scalar(lo0[:], hi0_ps[:], 20001.0, None, op0=Alu.subtract)
            thr = _emit_bisect(nc, lp, psb, s1, ones_bf, lo0, 20002.0, 24, float(K))

            # -- band = spans within EPS_BAND of the threshold (layout [16,480])
            s3 = sb.tile([16, NSP // 16], f32)
            nc.sync.dma_start(out=s3[:], in_=s_in.rearrange("(f p) -> p f", p=16))
            blo = sb.tile([128, 1], f32)
            bhi = sb.tile([128, 1], f32)
            nc.vector.tensor_scalar(blo[:], thr[:], EPS_BAND, None, op0=Alu.subtract)
            nc.vector.tensor_scalar(bhi[:], thr[:], EPS_BAND, None, op0=Alu.add)
            blo, bhi = blo[0:16, :], bhi[0:16, :]
            bm = sb.tile([16, NSP // 16], f32)
            bm2 = sb.tile([16, NSP // 16], f32)
            nc.vector.tensor_scalar(bm[:], s3[:], blo[:], None, op0=Alu.is_ge)
            nc.vector.tensor_scalar(bm2[:], s3[:], bhi[:], None, op0=Alu.is_le)
            nc.vector.tensor_tensor(out=bm[:], in0=bm[:], in1=bm2[:], op=Alu.mult)
            iota1 = sb.tile([16, NSP // 16], f32)
            nc.gpsimd.iota(iota1[:], pattern=[[16, NSP // 16]], base=1, channel_multiplier=1,
                           allow_small_or_imprecise_dtypes=True)
            bidx = sb.tile([16, NSP // 16 + 8], f32)
            nc.vector.tensor_tensor(out=bidx[:, :NSP // 16], in0=iota1[:], in1=bm[:], op=Alu.mult)
            nc.vector.tensor_scalar(bidx[:, :NSP // 16], bidx[:, :NSP // 16], 1.0, None, op0=Alu.subtract)
            nc.vector.memset(bidx[:, NSP // 16:], SENT)
            bcg = sb.tile([16, 8], f32)
            bnf = sb.tile([1, 1], u32)
            nc.gpsimd.sparse_gather(bcg[:], bidx[:], num_found=bnf[:])
            bti = sb.tile([16, 8], i32)
            nc.vector.tensor_copy(out=bti[:], in_=bcg[:])
            bscr = dp.tile([128], i32)
            nc.sync.dma_start(out=bscr[:].rearrange("(f p) -> p f", p=16), in_=bti[:])
            bIdx = sb.tile([128, 1], i32)
            nc.sync.dma_start(out=bIdx[:], in_=bscr[:].rearrange("(p o) -> p o", o=1))

            # -- gather band rows of X, transpose, fp32 re-score
            xb = gp.tile([128, D], f32, tag="xb")
            nc.gpsimd.indirect_dma_start(
                out=xb[:], out_offset=None, in_=x_in[:],
                in_offset=bass.IndirectOffsetOnAxis(ap=bIdx[:, 0:1], axis=0),
                bounds_check=NSP - 1, oob_is_err=False)
            ident = sb.tile([128, 128], f32)
            ones = sb.tile([128, 128], f32)
            nc.vector.memset(ones[:], 1.0)
            nc.gpsimd.affine_select(ident[:], ones[:], pattern=[[-1, 128]],
                                    compare_op=Alu.is_equal, fill=0.0, base=0, channel_multiplier=1)
            xtb = gp.tile([128, KT1, 128], f32, tag="xtb")
            for k in range(KT1):
                tp_ps = psp.tile([128, 128], f32, tag="tp")
                nc.tensor.transpose(tp_ps[:], xb[:, k * 128:(k + 1) * 128], ident[:])
                nc.vector.tensor_copy(out=xtb[:, k, :], in_=tp_ps[:])
            h1b = gp.tile([128, MT, 128], f32, tag="h1b")
            for m in range(MT):
                acc = psp2.tile([128, 128], f32, tag="bl1")
                for k in range(KT1):
                    nc.tensor.matmul(acc[:], w1_t[:, k, m * 128:(m + 1) * 128], xtb[:, k, :],
                                     start=(k == 0), stop=(k == KT1 - 1))
                nc.scalar.activation(h1b[:, m, :], acc[:], Act.Relu, bias=b1_t[:, m:m + 1])
            h2b = gp.tile([128, MT, 128], f32, tag="h2b")
            for g in range(MT):
                acc = psp.tile([128, 128], f32, tag="bl2")
                for k in range(KT2):
                    nc.tensor.matmul(acc[:], w2_t[:, k, g * 128:(g + 1) * 128], h1b[:, k, :],
                                     start=(k == 0), stop=(k == KT2 - 1))
                nc.scalar.activation(h2b[:, g, :], acc[:], Act.Relu, bias=b2_t[:, g:g + 1])
            acc3 = psp.tile([1, 128], f32, tag="bl3")
            for k in range(KT2):
                nc.tensor.matmul(acc3[:], w3_t[:, k:k + 1], h2b[:, k, :],
                                 start=(k == 0), stop=(k == KT2 - 1))
            sb_band = sb.tile([1, 128], f32)
            nc.vector.tensor_scalar(sb_band[:], acc3[:], b3_t[:], None, op0=Alu.add)
            # scatter needs values per partition: transpose [1,128] -> [128,1]
            sbt_ps = psp.tile([128, 1], f32, tag="bl3")
            nc.tensor.transpose(sbt_ps[:], sb_band[:], ident[0:1, 0:1])
            sbv = sb.tile([128, 1], f32)
            nc.vector.tensor_copy(out=sbv[:], in_=sbt_ps[:])

            # -- patch scores in DRAM
            s_patch = dp.tile([NSP], f32)
            nc.sync.dma_start(out=s_patch[:], in_=s_in[:])
            nc.gpsimd.indirect_dma_start(
                out=s_patch[:].rearrange("(n o) -> n o", o=1),
                out_offset=bass.IndirectOffsetOnAxis(ap=bIdx[:, 0:1], axis=0),
                in_=sbv[:], in_offset=None,
                bounds_check=NSP - 1, oob_is_err=False)
            nc.sync.dma_start(out=scores_out[:], in_=s_patch[:])

            # -- final threshold + selection on patched scores
            s1p = sb.tile([128, NSP // 128], f32)
            nc.sync.dma_start(out=s1p[:], in_=s_patch[:].rearrange("(p f) -> p f", p=128))
            lo2 = sb.tile([128, 1], f32)
            nc.vector.tensor_scalar(lo2[:], thr[:], 0.02, None, op0=Alu.subtract)
            thrf = _emit_bisect(nc, lp, psb, s1p, ones_bf, lo2, 0.04, 18, float(K))
            s3p = sb.tile([16, NSP // 16], f32)
            nc.sync.dma_start(out=s3p[:], in_=s_patch[:].rearrange("(f p) -> p f", p=16))
            m3 = sb.tile([16, NSP // 16], f32)
            nc.vector.tensor_scalar(m3[:], s3p[:], thrf[0:16, :], None, op0=Alu.is_ge)
            midx = sb.tile([16, NSP // 16 + 16], f32)
            nc.vector.tensor_tensor(out=midx[:, :NSP // 16], in0=iota1[:], in1=m3[:], op=Alu.mult)
            nc.vector.tensor_scalar(midx[:, :NSP // 16], midx[:, :NSP // 16], 1.0, None, op0=Alu.subtract)
            nc.vector.memset(midx[:, NSP // 16:], SENT)
            cg = sb.tile([16, 16], f32)
            nf = sb.tile([1, 1], u32)
            nc.gpsimd.sparse_gather(cg[:], midx[:], num_found=nf[:])
            ti = sb.tile([16, 16], i32)
            nc.vector.tensor_copy(out=ti[:], in_=cg[:])
            nc.sync.dma_start(out=tidx_out.rearrange("(f p) -> p f", p=16), in_=ti[:])
            tscr = dp.tile([256], i32)
            nc.sync.dma_start(out=tscr[:].rearrange("(f p) -> p f", p=16), in_=ti[:])
            idxA = sb.tile([128, 1], i32)
            idxB = sb.tile([128, 1], i32)
            nc.sync.dma_start(out=idxA[:], in_=tscr[0:128].rearrange("(p o) -> p o", o=1))
            nc.sync.dma_start(out=idxB[:], in_=tscr[128:256].rearrange("(p o) -> p o", o=1))

            # -- gathers
            for half, idx in ((0, idxA), (1, idxB)):
                fv = gp.tile([128, D], f32, tag="fv")
                nc.gpsimd.indirect_dma_start(
                    out=fv[:], out_offset=None, in_=x_in[:],
                    in_offset=bass.IndirectOffsetOnAxis(ap=idx, axis=0),
                    bounds_check=NSP - 1, oob_is_err=False)
                nc.sync.dma_start(out=fvec_out[half * 128:(half + 1) * 128, :], in_=fv[:])
                fs = gp.tile([128, 1], f32, tag="fs")
                nc.gpsimd.indirect_dma_start(
                    out=fs[:], out_offset=None, in_=s_patch[:].rearrange("(n o) -> n o", o=1),
                    in_offset=bass.IndirectOffsetOnAxis(ap=idx, axis=0),
                    bounds_check=NSP - 1, oob_is_err=False)
                nc.sync.dma_start(out=fs_out[half * 128:(half + 1) * 128, :], in_=fs[:])
                fb = gp.tile([128, 2], i32, tag="fb")
                nc.gpsimd.indirect_dma_start(
                    out=fb[:], out_offset=None, in_=bt[:],
                    in_offset=bass.IndirectOffsetOnAxis(ap=idx, axis=0),
                    bounds_check=NSP - 1, oob_is_err=False)
                nc.sync.dma_start(out=fbt_out[half * 128:(half + 1) * 128, :], in_=fb[:])

            # -- span length + masks
            sl_i = sb.tile([1, 1], i32)
            nc.sync.dma_start(out=sl_i[:], in_=seq[:])
            sl_f = sb.tile([1, 1], f32)
            nc.vector.tensor_copy(out=sl_f[:], in_=sl_i[:])
            nc.vector.tensor_scalar(sl_f[:], sl_f[:], 0.4, 0.49, op0=Alu.mult, op1=Alu.add)
            slq_i = sb.tile([1, 1], i32)
            nc.vector.tensor_copy(out=slq_i[:], in_=sl_f[:])  # round-to-nearest == ceil(0.4n)
            nc.sync.dma_start(out=sl_out[:], in_=slq_i[:])
            slq_f = sb.tile([1, 1], f32)
            nc.vector.tensor_copy(out=slq_f[:], in_=slq_i[:])
            iota_k = sb.tile([1, K], f32)
            nc.gpsimd.iota(iota_k[:], pattern=[[1, K]], base=0, channel_multiplier=0,
                           allow_small_or_imprecise_dtypes=True)
            valid = sb.tile([1, K], f32)
            nc.vector.tensor_scalar(valid[:], iota_k[:], slq_f[:], None, op0=Alu.is_lt)
            sqA_ps = psp.tile([128, K], f32, tag="tp")
            sqB_ps = psp.tile([77, K], f32, tag="bl2")
            nc.tensor.matmul(sqA_ps[:], valid[:, 0:128], valid[:], start=True, stop=True)
            nc.tensor.matmul(sqB_ps[:], valid[:, 128:K], valid[:], start=True, stop=True)
            sqA = sb.tile([128, K], f32)
            sqB = sb.tile([77, K], f32)
            nc.vector.tensor_copy(out=sqA[:], in_=sqA_ps[:])
            nc.vector.tensor_copy(out=sqB[:], in_=sqB_ps[:])
            nc.sync.dma_start(out=sq_out[0:128, :], in_=sqA[:])
            nc.sync.dma_start(out=sq_out[128:K, :], in_=sqB[:])
            triA = sb.tile([128, K], f32)
            triB = sb.tile([77, K], f32)
            nc.gpsimd.affine_select(triA[:], sqA[:], pattern=[[-1, K]], compare_op=Alu.is_ge,
                                    fill=0.0, base=0, channel_multiplier=1)
            nc.gpsimd.affine_select(triB[:], sqB[:], pattern=[[-1, K]], compare_op=Alu.is_ge,
                                    fill=0.0, base=128, channel_multiplier=1)
            nc.sync.dma_start(out=tri_out[0:128, :], in_=triA[:])
            nc.sync.dma_start(out=tri_out[128:K, :], in_=triB[:])
    nc.compile()
    return nc


# ------------------------------------------------------------------ PJRT runner
class _Runner:
    """Build a jitted sharded executor for a finalized Bass module once."""

    def __init__(self, nc, n_cores):
        import jax
        from jax.sharding import Mesh, PartitionSpec
        from jax.experimental.shard_map import shard_map
        from concourse.bass2jax import (_bass_exec_p, install_neuronx_cc_hook,
                                        partition_id_tensor)
        install_neuronx_cc_hook()
        self.jax = jax
        self.n_cores = n_cores
        partition_name = nc.partition_id_tensor.name if nc.partition_id_tensor else None
        in_names, out_names, out_avals, zero_outs = [], [], [], []
        for alloc in nc.m.functions[0].allocations:
            if not isinstance(alloc, mybir.MemoryLocationSet):
                continue
            name = alloc.memorylocations[0].name
            if alloc.kind == "ExternalInput":
                if name != partition_name:
                    in_names.append(name)
            elif alloc.kind == "ExternalOutput":
                out_names.append(name)
                out_avals.append(jax.core.ShapedArray(tuple(alloc.tensor_shape),
                                                      mybir.dt.np(alloc.dtype)))
                zero_outs.append(np.zeros(tuple(alloc.tensor_shape), mybir.dt.np(alloc.dtype)))
        self.in_names, self.out_names = in_names, out_names
        self.out_avals, self.zero_outs = out_avals, zero_outs
        all_in_names = list(in_names) + list(out_names)
        if partition_name is not None:
            all_in_names.append(partition_name)
        n_params, n_outs = len(in_names), len(out_avals)

        def _body(*args):
            operands = list(args)
            if partition_name is not None:
                operands.append(partition_id_tensor())
            return tuple(_bass_exec_p.bind(
                *operands,
                out_avals=tuple(out_avals), in_names=tuple(all_in_names),
                out_names=tuple(out_names), lowering_input_output_aliases=(),
                sim_require_finite=True, sim_require_nnan=True, nc=nc))

        devices = jax.devices()[:n_cores]
        self.mesh = Mesh(np.asarray(devices), ("core",))
        in_specs = (PartitionSpec("core"),) * (n_params + n_outs)
        out_specs = (PartitionSpec("core"),) * n_outs
        self.fn = jax.jit(shard_map(_body, mesh=self.mesh, in_specs=in_specs,
                                    out_specs=out_specs, check_rep=False),
                          keep_unused=True)

    def __call__(self, in_maps):
        n = self.n_cores
        args = [np.concatenate([np.asarray(in_maps[c][name]) for c in range(n)], axis=0)
                for name in self.in_names]
        args += [np.zeros((n * z.shape[0], *z.shape[1:]), z.dtype) for z in self.zero_outs]
        outs = self.fn(*args)
        self.jax.block_until_ready(outs)
        res = []
        for c in range(n):
            res.append({name: np.asarray(outs[i]).reshape(n, *self.out_avals[i].shape)[c]
                        for i, name in enumerate(self.out_names)})
        return res


def _get_runners():
    if "runners" not in _CACHE:
        nc1, nc2 = _build_nc1(), _build_nc2()
        _CACHE["nc1"], _CACHE["nc2"] = nc1, nc2
        _CACHE["runners"] = (_Runner(nc1, N_CORES), _Runner(nc2, N_CORES))
    return _CACHE["runners"]


# ----------------------------------------------------------------------- kernel
def kernel(span_vecs, span_mask, span_begin, span_end, sequence_lengths,
           W1, b1, W2, b2, W3, b3):
    span_vecs = np.asarray(span_vecs, np.float32)
    span_mask = np.asarray(span_mask, np.float32)
    span_begin = np.asarray(span_begin, np.int32)
    span_end = np.asarray(span_end, np.int32)
    sequence_lengths = np.asarray(sequence_lengths, np.int32)
    W1 = np.asarray(W1, np.float32)
    W2 = np.asarray(W2, np.float32)
    W3 = np.asarray(W3, np.float32)
    b1 = np.asarray(b1, np.float32)
    b2 = np.asarray(b2, np.float32)
    b3 = np.asarray(b3, np.float32)

    run1, run2 = _get_runners()
    wmap = {"w1": W1, "w2": W2, "w3": W3, "b1": b1, "b2": b2, "b3": b3}

    # ---- launch 1: MLP scores, data-parallel over (doc, half)
    in1 = []
    for c in range(N_CORES):
        d, h = c // 2, c % 2
        xh = span_vecs[d].reshape(NSP, D)[h * NSPH:(h + 1) * NSPH]
        in1.append({"xt": np.ascontiguousarray(xh.T),
                    "msk": span_mask[d].reshape(NSP)[h * NSPH:(h + 1) * NSPH],
                    **wmap})
    res1 = run1(in1)

    # ---- host reshard: full per-doc score vectors
    s_full = [np.concatenate([res1[2 * d]["s_half"], res1[2 * d + 1]["s_half"]])
              for d in range(B)]

    # ---- launch 2: selection (both cores of a pair run the identical doc)
    in2 = []
    for c in range(N_CORES):
        d = c // 2
        bt = np.stack([span_begin[d].reshape(NSP), span_end[d].reshape(NSP)], axis=1)
        in2.append({"s_in": s_full[d],
                    "x_in": span_vecs[d].reshape(NSP, D),
                    "seq": sequence_lengths[d].reshape(1, 1),
                    "bt": np.ascontiguousarray(bt),
                    **wmap})
    res2 = run2(in2)

    # ---- unshard
    prune_scores = np.stack([res2[2 * d]["scores_out"] for d in range(B)]
                            ).reshape(B, T, L, 1)
    top_idx = np.stack([res2[2 * d]["tidx_out"][:K] for d in range(B)])
    f_vecs = np.stack([res2[2 * d]["fvec_out"][:K] for d in range(B)])
    f_scores = np.stack([res2[2 * d]["fs_out"][:K] for d in range(B)])
    f_begin = np.stack([res2[2 * d]["fbt_out"][:K, 0:1] for d in range(B)])
    f_end = np.stack([res2[2 * d]["fbt_out"][:K, 1:2] for d in range(B)])
    span_lengths = np.stack([res2[2 * d]["sl_out"][0, 0] for d in range(B)])
    square_mask = np.stack([res2[2 * d]["sq_out"] for d in range(B)])
    triangular_mask = np.stack([res2[2 * d]["tri_out"] for d in range(B)])

    return (prune_scores, top_idx.astype(np.int32), f_vecs, f_scores,
            f_begin.astype(np.int32), f_end.astype(np.int32),
            span_lengths.astype(np.int32), square_mask, triangular_mask)
